# revision 18
# baseline (speedup 1.0000x reference)
"""Positional-encoding kernel for Trainium2 (8 NeuronCores).

The reference output [32, 4096, 2048] f32 is a batch-broadcast of the
interleaved sin/cos PE table [4096, 2048]; it does not depend on x.
Sharding: by sequence -- core i computes table rows [512*i, 512*(i+1))
on-device and stores exactly its 4 MiB shard (the unique output
content); the batch broadcast is host-side unshard.  (v1 wrote the
full 1 GiB from the device at ~392 us, the HBM-write roofline for that
strategy; v2 loaded+stored the table at ~33 us; this version computes
it on-device with all engines.)

Measured machine model driving the design (NTFF traces):
  - any DMA pays a per-queue engine ramp (~250 ns per chunk, P>=16 ->
    16 chunks ~4 us), so the only input is one tiny P=8 descriptor;
  - DVE [128,1024] op ~0.7 us SBUF / ~1.2 us reading PSUM; GpSimd is
    2-4x slower than DVE but idle otherwise; ACT Sin ~1.2 us; PE fp32
    matmul is 4x slower than bf16; ACT reading PSUM crashes at runtime
    (walrus compiles it; bisected on HW), so ACT only reads SBUF;
  - custom-DVE ops (ADD_RANGE_WRAP etc.) fail codegen in this walrus.

Per core, 4 row groups of 128 rows (partition p = row in group):
  PE    u_g = pos (x) freq2 (turns) -> PSUM [128,1024] per group, via
        exact bf16 splits (pos = a + b, a a multiple of 16; freq2 =
        fh+fm+fl ~24 bits), K=6 outer product, 2x N=512 matmuls.
  DVE   k    = (u + 1.5*2^23) - 1.5*2^23   fused ts = round-nearest
        y_s  = u - k    in [-.5,.5]  (turns; period is exactly 1, and
                                      for high k, u<0.5 passes through)
        ya   = |y_s|  via one bitcast-u32 ts bitwise_and 0x7fffffff
  ACT   even cols: Sin(2pi*y_s)              = sin(theta)
        odd  cols: Sin(-2pi*|y_s| + pi/2)    = cos(theta), arg lands
        in [-pi/2, pi/2], the Sin table's best-accuracy zone
        (HW Sin is only accurate on [-pi,pi]; the Sin table is
        prefetched by a dummy op at block entry)
ACT writes separate contiguous sin/cos bf16 blocks (no stride-2 write
tax; the host interleaves + upcasts during unshard).  Stores are 8x
256 KiB descriptors, each gated on its own ACT op, so sin blocks fly
~1.4 us before their cos sibling.  Group 0 is processed in two column
halves so ACT starts right after the first PE matmul.  Max abs err
~2.5e-3 (gate 1e-2).
"""

import math

import numpy as np

SEQ = 4096
D = 2048
B = 32
N_CORES = 8
S_SHARD = SEQ // N_CORES          # 512
NG = 4                            # row groups of 128 per core

C_MAGIC = 12582912.0              # 1.5 * 2^23
PI = math.pi
TWO_PI = 2.0 * math.pi

_cache = {}


def _pe_table() -> np.ndarray:
    pos = np.arange(SEQ, dtype=np.float64)[:, None]
    k = np.arange(D // 2, dtype=np.float64)[None, :]
    theta = pos * np.power(10000.0, -k / (D // 2))
    pe = np.stack([np.sin(theta), np.cos(theta)], axis=-1)
    return pe.reshape(SEQ, D).astype(np.float32)


def build_nc():
    import concourse.bass as bass
    import concourse.mybir as mybir
    from contextlib import ExitStack

    f32 = mybir.dt.float32
    u32 = mybir.dt.uint32
    bf16 = mybir.dt.bfloat16
    M = mybir.AluOpType
    Sin = mybir.ActivationFunctionType.Sin

    nc = bass.Bass()
    # aux rows 0..5, cols 0:1024:    [fh|fm|fl|fh|fm|fl](k)
    # aux rows 0..5, cols 1024+128g: lhsT rows [a_g,a_g,a_g,b,b,b]
    aux_in = nc.dram_tensor("aux", [8, 1536], bf16, kind="ExternalInput")
    outS = nc.dram_tensor("outS", [S_SHARD, D // 2], bf16, kind="ExternalOutput")
    outC = nc.dram_tensor("outC", [S_SHARD, D // 2], bf16, kind="ExternalOutput")

    es = ExitStack()
    T = lambda nm, sh, dt: es.enter_context(nc.sbuf_tensor(nm, list(sh), dt))
    aux = T("aux_s", (8, 1536), bf16)
    tileS = T("tileS", (128, NG * 1024), bf16)
    tileC = T("tileC", (128, NG * 1024), bf16)
    ys = T("ys", (128, NG * 1024), f32)
    yc = T("yc", (128, NG * 1024), f32)
    kk = T("kk", (128, 1024), f32)
    bias0 = T("bias0", (128, 1), f32)
    half = T("half", (128, 1), f32)
    scr = T("scr", (128, 1), f32)
    up = [
        es.enter_context(nc.psum_tensor(f"u{g}", [128, 1024], f32))
        for g in range(NG)
    ]
    ld = es.enter_context(nc.semaphore("ld"))
    bs = es.enter_context(nc.semaphore("bs"))
    pe = es.enter_context(nc.semaphore("pe"))
    yss = es.enter_context(nc.semaphore("yss"))
    ysm = es.enter_context(nc.semaphore("ysm"))
    acts = es.enter_context(nc.semaphore("acts"))
    actc = es.enter_context(nc.semaphore("actc"))
    st = es.enter_context(nc.semaphore("st"))

    with nc.Block() as block:
        def seg(buf, g):
            return buf[:, g * 1024 : (g + 1) * 1024]

        @block.tensor
        def _(tensor):
            tensor.wait_ge(ld, 16)
            for g in range(NG):
                lhsT = aux[0:6, 1024 + 128 * g : 1024 + 128 * (g + 1)]
                for j in (0, 1):
                    tensor.matmul(
                        up[g][:, j * 512 : (j + 1) * 512],
                        lhsT,
                        aux[0:6, j * 512 : (j + 1) * 512],
                        start=True, stop=True,
                    ).then_inc(pe, 1)

        @block.gpsimd
        def _(gpsimd):
            # aux load issued from GpSimd's queue: it clears its entry
            # drain ~0.5 us before Sync does
            gpsimd.dma_start(out=aux[:, :], in_=aux_in[:, :]).then_inc(ld, 16)
            gpsimd.memset(bias0[:, :], 0.0)
            gpsimd.memset(half[:, :], PI / 2.0).then_inc(bs, 1)

        @block.vector
        def _(vector):
            ts = vector.tensor_scalar
            # group 0 in two column halves so ACT can start ~1.4 us earlier
            for j in (0, 1):
                h = slice(j * 512, (j + 1) * 512)
                vector.wait_ge(pe, j + 1)
                ts(kk[:, h], up[0][:, h], C_MAGIC, C_MAGIC, M.add, M.subtract)
                vector.tensor_tensor(
                    out=ys[:, h], in0=up[0][:, h], in1=kk[:, h],
                    op=M.subtract,
                ).then_inc(yss, 1)
                ts(yc[:, h].bitcast(u32), ys[:, h].bitcast(u32),
                   0x7FFFFFFF, None, M.bitwise_and).then_inc(ysm, 1)
            for g in range(1, NG):
                vector.wait_ge(pe, 2 * (g + 1))
                ts(kk[:, :], up[g][:, :], C_MAGIC, C_MAGIC, M.add, M.subtract)
                vector.tensor_tensor(
                    out=seg(ys, g), in0=up[g][:, :], in1=kk[:, :],
                    op=M.subtract,
                ).then_inc(yss, 1)
                ts(seg(yc, g).bitcast(u32), seg(ys, g).bitcast(u32),
                   0x7FFFFFFF, None, M.bitwise_and).then_inc(ysm, 1)

        @block.scalar
        def _(scalar):
            # dummy op pulls the Sin table in at block entry
            scalar.wait_ge(bs, 1)
            scalar.activation(scr[:, :], bias0[:, :], Sin, bias=bias0[:, 0:1])
            for j in (0, 1):
                h = slice(j * 512, (j + 1) * 512)
                scalar.wait_ge(yss, j + 1)
                a = scalar.activation(
                    tileS[:, h], ys[:, h], Sin, bias=bias0[:, 0:1],
                    scale=TWO_PI,
                )
                if j == 1:
                    a.then_inc(acts, 1)
                scalar.wait_ge(ysm, j + 1)
                a = scalar.activation(
                    tileC[:, h], yc[:, h], Sin, bias=half[:, 0:1],
                    scale=-TWO_PI,
                )
                if j == 1:
                    a.then_inc(actc, 1)
            for g in range(1, NG):
                scalar.wait_ge(yss, g + 2)
                scalar.activation(
                    seg(tileS, g), seg(ys, g), Sin, bias=bias0[:, 0:1],
                    scale=TWO_PI,
                ).then_inc(acts, 1)
                scalar.wait_ge(ysm, g + 2)
                scalar.activation(
                    seg(tileC, g), seg(yc, g), Sin, bias=half[:, 0:1],
                    scale=-TWO_PI,
                ).then_inc(actc, 1)

        @block.sync
        def _(sync):
            for g in range(NG):
                sync.wait_ge(acts, g + 1)
                sync.dma_start(
                    out=outS[g * 128 : (g + 1) * 128, :],
                    in_=seg(tileS, g),
                ).then_inc(st, 16)
                sync.wait_ge(actc, g + 1)
                sync.dma_start(
                    out=outC[g * 128 : (g + 1) * 128, :],
                    in_=seg(tileC, g),
                ).then_inc(st, 16)
            sync.wait_ge(st, 16 * 2 * NG)

    es.close()
    return nc


def make_in_maps(pe_unused=None):
    import concourse.mybir as mybir

    bf16 = mybir.dt.np(mybir.dt.bfloat16)

    k = np.arange(1024, dtype=np.float64)
    freq2 = 1.0 / (2.0 * np.pi * np.power(10000.0, k / 1024.0))  # f64 turns
    fh = freq2.astype(bf16)
    fm = (freq2 - fh.astype(np.float64)).astype(bf16)
    fl = (freq2 - fh.astype(np.float64) - fm.astype(np.float64)).astype(bf16)

    p = np.arange(128, dtype=np.float64)
    b = np.mod(p, 16.0)                       # exact in bf16
    maps = []
    for i in range(N_CORES):
        aux = np.zeros((8, 1536), dtype=bf16)
        for r, f in zip(range(6), (fh, fm, fl, fh, fm, fl)):
            aux[r, 0:1024] = f
        for g in range(NG):
            s0 = 512.0 * i + 128.0 * g
            a = s0 + 16.0 * np.floor(p / 16.0)  # multiple of 16 -> exact bf16
            cols = slice(1024 + 128 * g, 1024 + 128 * (g + 1))
            for r in (0, 1, 2):
                aux[r, cols] = a.astype(bf16)
            for r in (3, 4, 5):
                aux[r, cols] = b.astype(bf16)
        maps.append({"aux": aux})
    return maps


def assemble(results) -> np.ndarray:
    """results: per-core dicts with outS/outC bf16 blocks."""
    s = np.concatenate([r["outS"] for r in results], axis=0)
    c = np.concatenate([r["outC"] for r in results], axis=0)
    pe = np.empty((SEQ, D), dtype=np.float32)
    pe[:, 0::2] = s.astype(np.float32)
    pe[:, 1::2] = c.astype(np.float32)
    full = np.empty((B, SEQ, D), dtype=np.float32)
    full[:] = pe[None, :, :]
    return full


def kernel(x: np.ndarray) -> np.ndarray:
    from concourse.bass_utils import run_bass_kernel_spmd

    assert x.shape[0] == B

    if "nc" not in _cache:
        _cache["nc"] = build_nc()
    res = run_bass_kernel_spmd(
        _cache["nc"], make_in_maps(), list(range(N_CORES))
    )
    return assemble([res.results[i] for i in range(N_CORES)])


# revision 19
# speedup vs baseline: 1.0158x; 1.0158x over previous
"""Positional-encoding kernel for Trainium2 (8 NeuronCores).

The reference output [32, 4096, 2048] f32 is a batch-broadcast of the
interleaved sin/cos PE table [4096, 2048]; it does not depend on x.
Sharding: by sequence -- core i computes table rows [512*i, 512*(i+1))
on-device and stores exactly its 4 MiB shard (the unique output
content); the batch broadcast is host-side unshard.  (v1 wrote the
full 1 GiB from the device at ~392 us, the HBM-write roofline for that
strategy; v2 loaded+stored the table at ~33 us; this version computes
it on-device with all engines.)

Measured machine model driving the design (NTFF traces):
  - any DMA pays a per-queue engine ramp (~250 ns per chunk, P>=16 ->
    16 chunks ~4 us), so the only input is one tiny P=8 descriptor;
  - DVE [128,1024] op ~0.7 us SBUF / ~1.2 us reading PSUM; GpSimd is
    2-4x slower than DVE but idle otherwise; ACT Sin ~1.2 us; PE fp32
    matmul is 4x slower than bf16; ACT reading PSUM crashes at runtime
    (walrus compiles it; bisected on HW), so ACT only reads SBUF;
  - custom-DVE ops (ADD_RANGE_WRAP etc.) fail codegen in this walrus.

Per core, 4 row groups of 128 rows (partition p = row in group):
  PE    u_g = pos (x) freq2 (turns) -> PSUM [128,1024] per group, via
        exact bf16 splits (pos = a + b, a a multiple of 16; freq2 =
        fh+fm+fl ~24 bits), K=6 outer product, 2x N=512 matmuls.
  DVE   k    = (u + 1.5*2^23) - 1.5*2^23   fused ts = round-nearest
        y_s  = u - k    in [-.5,.5]  (turns; period is exactly 1, and
                                      for high k, u<0.5 passes through)
        ya   = |y_s|  via one bitcast-u32 ts bitwise_and 0x7fffffff
  ACT   even cols: Sin(2pi*y_s)              = sin(theta)
        odd  cols: Sin(-2pi*|y_s| + pi/2)    = cos(theta), arg lands
        in [-pi/2, pi/2], the Sin table's best-accuracy zone
        (HW Sin is only accurate on [-pi,pi]; the Sin table is
        prefetched by a dummy op at block entry)
ACT writes separate contiguous sin/cos bf16 blocks (no stride-2 write
tax; the host interleaves + upcasts during unshard).  Stores are 8x
256 KiB descriptors, each gated on its own ACT op, so sin blocks fly
~1.4 us before their cos sibling.  Group 0 is processed in two column
halves so ACT starts right after the first PE matmul.  Max abs err
~2.5e-3 (gate 1e-2).
"""

import math

import numpy as np

SEQ = 4096
D = 2048
B = 32
N_CORES = 8
S_SHARD = SEQ // N_CORES          # 512
NG = 4                            # row groups of 128 per core

C_MAGIC = 12582912.0              # 1.5 * 2^23
PI = math.pi
TWO_PI = 2.0 * math.pi

_cache = {}


def _pe_table() -> np.ndarray:
    pos = np.arange(SEQ, dtype=np.float64)[:, None]
    k = np.arange(D // 2, dtype=np.float64)[None, :]
    theta = pos * np.power(10000.0, -k / (D // 2))
    pe = np.stack([np.sin(theta), np.cos(theta)], axis=-1)
    return pe.reshape(SEQ, D).astype(np.float32)


def build_nc():
    import concourse.bass as bass
    import concourse.mybir as mybir
    from contextlib import ExitStack

    f32 = mybir.dt.float32
    u32 = mybir.dt.uint32
    bf16 = mybir.dt.bfloat16
    M = mybir.AluOpType
    Sin = mybir.ActivationFunctionType.Sin

    nc = bass.Bass()
    # aux rows 0..5, cols 0:1024:    [fh|fm|fl|fh|fm|fl](k)
    # aux rows 0..5, cols 1024+128g: lhsT rows [a_g,a_g,a_g,b,b,b]
    aux_in = nc.dram_tensor("aux", [8, 1536], bf16, kind="ExternalInput")
    outS = nc.dram_tensor("outS", [S_SHARD, D // 2], bf16, kind="ExternalOutput")
    outC = nc.dram_tensor("outC", [S_SHARD, D // 2], bf16, kind="ExternalOutput")

    es = ExitStack()
    T = lambda nm, sh, dt: es.enter_context(nc.sbuf_tensor(nm, list(sh), dt))
    aux = T("aux_s", (8, 1536), bf16)
    tileS = T("tileS", (128, NG * 1024), bf16)
    tileC = T("tileC", (128, NG * 1024), bf16)
    ys = T("ys", (128, NG * 1024), f32)
    yc = T("yc", (128, NG * 1024), f32)
    kk = T("kk", (128, 1024), f32)
    bias0 = T("bias0", (128, 1), f32)
    half = T("half", (128, 1), f32)
    scr = T("scr", (128, 1), f32)
    up = [
        es.enter_context(nc.psum_tensor(f"u{g}", [128, 1024], f32))
        for g in range(NG)
    ]
    ld = es.enter_context(nc.semaphore("ld"))
    bs = es.enter_context(nc.semaphore("bs"))
    pe = es.enter_context(nc.semaphore("pe"))
    yss = es.enter_context(nc.semaphore("yss"))
    ysm = es.enter_context(nc.semaphore("ysm"))
    acts = es.enter_context(nc.semaphore("acts"))
    actc = es.enter_context(nc.semaphore("actc"))
    st = es.enter_context(nc.semaphore("st"))

    with nc.Block() as block:
        def seg(buf, g):
            return buf[:, g * 1024 : (g + 1) * 1024]

        @block.tensor
        def _(tensor):
            tensor.wait_ge(ld, 16)
            for g in range(NG):
                lhsT = aux[0:6, 1024 + 128 * g : 1024 + 128 * (g + 1)]
                for j in (0, 1):
                    tensor.matmul(
                        up[g][:, j * 512 : (j + 1) * 512],
                        lhsT,
                        aux[0:6, j * 512 : (j + 1) * 512],
                        start=True, stop=True,
                    ).then_inc(pe, 1)

        @block.gpsimd
        def _(gpsimd):
            gpsimd.memset(bias0[:, :], 0.0)
            gpsimd.memset(half[:, :], PI / 2.0).then_inc(bs, 1)

        @block.vector
        def _(vector):
            ts = vector.tensor_scalar
            # group 0 in two column halves so ACT can start ~1.4 us earlier
            for j in (0, 1):
                h = slice(j * 512, (j + 1) * 512)
                vector.wait_ge(pe, j + 1)
                ts(kk[:, h], up[0][:, h], C_MAGIC, C_MAGIC, M.add, M.subtract)
                vector.tensor_tensor(
                    out=ys[:, h], in0=up[0][:, h], in1=kk[:, h],
                    op=M.subtract,
                ).then_inc(yss, 1)
                ts(yc[:, h].bitcast(u32), ys[:, h].bitcast(u32),
                   0x7FFFFFFF, None, M.bitwise_and).then_inc(ysm, 1)
            for g in range(1, NG):
                vector.wait_ge(pe, 2 * (g + 1))
                ts(kk[:, :], up[g][:, :], C_MAGIC, C_MAGIC, M.add, M.subtract)
                vector.tensor_tensor(
                    out=seg(ys, g), in0=up[g][:, :], in1=kk[:, :],
                    op=M.subtract,
                ).then_inc(yss, 1)
                ts(seg(yc, g).bitcast(u32), seg(ys, g).bitcast(u32),
                   0x7FFFFFFF, None, M.bitwise_and).then_inc(ysm, 1)

        @block.scalar
        def _(scalar):
            # dummy op pulls the Sin table in at block entry
            scalar.wait_ge(bs, 1)
            scalar.activation(scr[:, :], bias0[:, :], Sin, bias=bias0[:, 0:1])
            for j in (0, 1):
                h = slice(j * 512, (j + 1) * 512)
                scalar.wait_ge(yss, j + 1)
                a = scalar.activation(
                    tileS[:, h], ys[:, h], Sin, bias=bias0[:, 0:1],
                    scale=TWO_PI,
                )
                if j == 1:
                    a.then_inc(acts, 1)
                scalar.wait_ge(ysm, j + 1)
                a = scalar.activation(
                    tileC[:, h], yc[:, h], Sin, bias=half[:, 0:1],
                    scale=-TWO_PI,
                )
                if j == 1:
                    a.then_inc(actc, 1)
            for g in range(1, NG):
                scalar.wait_ge(yss, g + 2)
                scalar.activation(
                    seg(tileS, g), seg(ys, g), Sin, bias=bias0[:, 0:1],
                    scale=TWO_PI,
                ).then_inc(acts, 1)
                scalar.wait_ge(ysm, g + 2)
                scalar.activation(
                    seg(tileC, g), seg(yc, g), Sin, bias=half[:, 0:1],
                    scale=-TWO_PI,
                ).then_inc(actc, 1)

        @block.sync
        def _(sync):
            sync.dma_start(out=aux[:, :], in_=aux_in[:, :]).then_inc(ld, 16)
            for g in range(NG):
                sync.wait_ge(acts, g + 1)
                sync.dma_start(
                    out=outS[g * 128 : (g + 1) * 128, :],
                    in_=seg(tileS, g),
                ).then_inc(st, 16)
                sync.wait_ge(actc, g + 1)
                sync.dma_start(
                    out=outC[g * 128 : (g + 1) * 128, :],
                    in_=seg(tileC, g),
                ).then_inc(st, 16)
            sync.wait_ge(st, 16 * 2 * NG)

    es.close()
    return nc


def make_in_maps(pe_unused=None):
    import concourse.mybir as mybir

    bf16 = mybir.dt.np(mybir.dt.bfloat16)

    k = np.arange(1024, dtype=np.float64)
    freq2 = 1.0 / (2.0 * np.pi * np.power(10000.0, k / 1024.0))  # f64 turns
    fh = freq2.astype(bf16)
    fm = (freq2 - fh.astype(np.float64)).astype(bf16)
    fl = (freq2 - fh.astype(np.float64) - fm.astype(np.float64)).astype(bf16)

    p = np.arange(128, dtype=np.float64)
    b = np.mod(p, 16.0)                       # exact in bf16
    maps = []
    for i in range(N_CORES):
        aux = np.zeros((8, 1536), dtype=bf16)
        for r, f in zip(range(6), (fh, fm, fl, fh, fm, fl)):
            aux[r, 0:1024] = f
        for g in range(NG):
            s0 = 512.0 * i + 128.0 * g
            a = s0 + 16.0 * np.floor(p / 16.0)  # multiple of 16 -> exact bf16
            cols = slice(1024 + 128 * g, 1024 + 128 * (g + 1))
            for r in (0, 1, 2):
                aux[r, cols] = a.astype(bf16)
            for r in (3, 4, 5):
                aux[r, cols] = b.astype(bf16)
        maps.append({"aux": aux})
    return maps


def assemble(results) -> np.ndarray:
    """results: per-core dicts with outS/outC bf16 blocks."""
    s = np.concatenate([r["outS"] for r in results], axis=0)
    c = np.concatenate([r["outC"] for r in results], axis=0)
    pe = np.empty((SEQ, D), dtype=np.float32)
    pe[:, 0::2] = s.astype(np.float32)
    pe[:, 1::2] = c.astype(np.float32)
    full = np.empty((B, SEQ, D), dtype=np.float32)
    full[:] = pe[None, :, :]
    return full


def kernel(x: np.ndarray) -> np.ndarray:
    from concourse.bass_utils import run_bass_kernel_spmd

    assert x.shape[0] == B

    if "nc" not in _cache:
        _cache["nc"] = build_nc()
    res = run_bass_kernel_spmd(
        _cache["nc"], make_in_maps(), list(range(N_CORES))
    )
    return assemble([res.results[i] for i in range(N_CORES)])
